# revision 13
# baseline (speedup 1.0000x reference)
# Distributed Trainium2 kernel for the dual-map spatial attention module:
#   x3 = x[:, :64], x2 = x[:, 64:]  (as [B, 64, N], N = 64*64 = 4096)
#   p2 = wq2 @ x2 + bq2 ; p3 = wq3 @ x3 + bq3 ; v3 = wv3 @ x3 + bv3
#   att32 = softmax(p3^T p2), att33 = softmax(p3^T p3)   (row softmax over keys)
#   out = gamma2 * (v3 @ att32^T) + gamma3 * (v3 @ att33^T) + x3
#
# Sharding: data-parallel over batch (4) x query-halves (2) -> 8 cores.
# Each core sees the full keys/values of its batch and computes the output
# for its 2048 query columns. No collectives needed.
#
# Per-core dataflow (all attention math in a key-streaming loop):
#   - project p2/p3 keys [8, 4096], queries p3q [8, 2048], and the
#     transposed value matrix v3T [4096, 65] (last column = ones, so the
#     out-matmul also produces the softmax denominator row).
#   - for each 512-wide query chunk, stream 32 key tiles of 128:
#       energy matmuls (K=8) -> PSUM f32, one Exp over [128, 1024]
#       (both attention maps share the activation op) -> bf16 SBUF,
#       immediately consumed by the accumulating out-matmuls (K=128).
#   - epilogue: reciprocal of the denominator row, gamma folded into a
#     rank-1 broadcast matmul, two fused multiplies + residual add.
import sys

if "/opt/trn_rl_repo" not in sys.path:
    sys.path.insert(0, "/opt/trn_rl_repo")

from contextlib import ExitStack

import numpy as np
import ml_dtypes

import concourse.bass as bass
import concourse.tile as tile
from concourse import bacc, mybir
from concourse.bass_utils import run_bass_kernel_spmd

BF16 = ml_dtypes.bfloat16
dt = mybir.dt

N = 4096          # keys per batch (64*64 spatial positions)
M_LOC = 2048      # queries per core (half a batch)
CH = 64           # output channels (c_half)
D = 8             # q/k projection dim
KA = CH + 1       # augmented contraction dim (channels + ones row)
NT = N // 128     # key tiles
MC = M_LOC // 512 # query chunks per core

def ts(i, size):
    return slice(i * size, (i + 1) * size)


def build(gamma2: float, gamma3: float) -> bass.Bass:
    # Bacc (vs raw Bass): its compile() splits multi-semaphore waits into
    # event-semaphore chains, which this walrus build requires (it rejects
    # instructions carrying more than one sync wait).
    nc = bacc.Bacc()

    x3aug = nc.declare_dram_parameter("x3aug", [KA, N], dt.bfloat16, isOutput=False)
    x2aug = nc.declare_dram_parameter("x2aug", [KA, N], dt.bfloat16, isOutput=False)
    x3q = nc.declare_dram_parameter("x3q", [KA, M_LOC], dt.bfloat16, isOutput=False)
    x3res = nc.declare_dram_parameter("x3res", [CH, M_LOC], dt.float32, isOutput=False)
    wq2bT = nc.declare_dram_parameter("wq2bT", [KA, D], dt.bfloat16, isOutput=False)
    wq3bT = nc.declare_dram_parameter("wq3bT", [KA, D], dt.bfloat16, isOutput=False)
    wv3bT = nc.declare_dram_parameter("wv3bT", [KA, KA], dt.bfloat16, isOutput=False)
    out_e = nc.declare_dram_parameter("out", [CH, M_LOC], dt.float32, isOutput=True)

    EXP = mybir.ActivationFunctionType.Exp
    NXC = 4               # x2aug/x3aug arrive in NXC column chunks
    XC = N // NXC

    with ExitStack() as ctx:
        tc = ctx.enter_context(tile.TileContext(nc))
        singles = ctx.enter_context(tc.tile_pool(name="singles", bufs=1))
        ps_e = ctx.enter_context(tc.tile_pool(name="ps_e", bufs=3, space="PSUM"))
        ps_o = ctx.enter_context(tc.tile_pool(name="ps_o", bufs=1, space="PSUM"))
        sb_e = ctx.enter_context(tc.tile_pool(name="sb_e", bufs=3))
        sb_tmp = ctx.enter_context(tc.tile_pool(name="sb_tmp", bufs=2))
        sb_out = ctx.enter_context(tc.tile_pool(name="sb_out", bufs=2))

        # ---- inputs -> SBUF.  Weights first (the projections need them
        # immediately and the HWDGE queue is in-order), then the activations
        # in first-use order.  x2aug/x3aug land as separate column-chunk
        # tiles so the just-in-time projections only wait for their chunk.
        wq2bT_sb = singles.tile([KA, D], dt.bfloat16)
        nc.sync.dma_start(out=wq2bT_sb, in_=wq2bT[:, :])
        wq3bT_sb = singles.tile([KA, D], dt.bfloat16)
        nc.sync.dma_start(out=wq3bT_sb, in_=wq3bT[:, :])
        wv3bT_sb = singles.tile([KA, KA], dt.bfloat16)
        nc.sync.dma_start(out=wv3bT_sb, in_=wv3bT[:, :])
        x3q_sb = singles.tile([KA, M_LOC], dt.bfloat16)
        nc.sync.dma_start(out=x3q_sb, in_=x3q[:, :])

        x2c = []
        x3c = []
        for c in range(NXC):
            t2 = singles.tile([KA, XC], dt.bfloat16, name=f"x2c{c}", tag=f"x2c{c}")
            nc.sync.dma_start(out=t2, in_=x2aug[:, ts(c, XC)])
            x2c.append(t2)
            t3 = singles.tile([KA, XC], dt.bfloat16, name=f"x3c{c}", tag=f"x3c{c}")
            nc.sync.dma_start(out=t3, in_=x3aug[:, ts(c, XC)])
            x3c.append(t3)

        # residual lands on rows 1:65 so every epilogue op is base-partition-0
        x3res_sb = singles.tile([KA, M_LOC], dt.float32)
        nc.vector.memset(x3res_sb[0:1, :], 0.0)
        nc.sync.dma_start(out=x3res_sb[1 : 1 + CH, :], in_=x3res[:, :])

        # Keys and queries live four times, at partition rows 0:8 / 32:40 /
        # 64:72 / 96:104 (p2, p3, p2, p3).  The four energy matmuls of a
        # step pair then run in the four disjoint 32-row PE strips (row
        # tiles T0/T4/T8/T12) concurrently.
        p_all = singles.tile([128, N], dt.bfloat16)
        q_all = singles.tile([128, M_LOC], dt.bfloat16)
        v3t = singles.tile([128, NT, KA], dt.bfloat16)

        def x_slice(tiles, j):
            # 512-column slice j out of the chunked x tiles
            per = XC // 512
            return tiles[j // per][:, ts(j % per, 512)]

        def proj_chunk(dst, j, lhs0, rhs0, lhs1, rhs1):
            # four column-packed projections into one PSUM tile (rows 0:8 and
            # 64:72 from (lhs0, rhs0); rows 32:40 and 96:104 from
            # (lhs1, rhs1)); one wide cast out.
            pp = ps_e.tile([128, 512], dt.float32, tag="e")
            for g, (lh, rh) in enumerate(
                ((lhs0, rhs0), (lhs1, rhs1), (lhs0, rhs0), (lhs1, rhs1))
            ):
                nc.tensor.matmul(
                    pp[32 * g : 32 * g + D, :], lhsT=lh, rhs=rh,
                    start=True, stop=True, tile_position=(0, 32 * g),
                )
            nc.vector.tensor_copy(out=dst[:, ts(j, 512)], in_=pp)

        def v3t_make(ntl):
            # v3T tile [128 keys, 65] = x3aug_tile^T @ wv3bT; column 0 of
            # wv3bT picks out the ones row -> out-matmul row 0 accumulates
            # the softmax denominator.
            vp = ps_e.tile([128, KA], dt.float32, tag="e")
            per = XC // 128
            nc.tensor.matmul(
                vp, lhsT=x3c[ntl // per][:, ts(ntl % per, 128)], rhs=wv3bT_sb,
                start=True, stop=True,
            )
            nc.vector.tensor_copy(out=v3t[:, ntl, :], in_=vp)

        def q_chunk(j):
            proj_chunk(q_all, j, wq3bT_sb, x3q_sb[:, ts(j, 512)],
                       wq3bT_sb, x3q_sb[:, ts(j, 512)])

        def p_chunk(j):
            proj_chunk(p_all, j, wq2bT_sb, x_slice(x2c, j),
                       wq3bT_sb, x_slice(x3c, j))

        # ---- main attention loop, software-pipelined: the out-matmuls of
        # step i are emitted next to the energy matmuls of step i+1, so the
        # (in-order) PE stream never parks right behind the Exp it feeds.
        # Key-side projections and v3T tiles are produced just-in-time
        # during the first query chunk; later query chunks are produced in
        # the middle of the preceding chunk's loop.
        o_tiles = {}

        def emit_stage(mc):
            # stage the accumulators out of PSUM quickly (frees the o banks
            # for the next chunk); the normalization itself is deferred a few
            # steps so its PE work never head-of-line-blocks the energy
            # matmuls while the reciprocals run on the vector engine.
            o32, o33 = o_tiles.pop(mc)
            s32 = sb_tmp.tile([KA, 512], dt.float32, tag="s32")
            nc.vector.tensor_copy(out=s32, in_=o32)
            s33 = sb_tmp.tile([KA, 512], dt.float32, tag="s33")
            nc.vector.tensor_copy(out=s33, in_=o33)
            r32 = sb_tmp.tile([1, 512], dt.float32, tag="r32")
            nc.vector.reciprocal_approx_fast(out=r32, in_=s32[0:1, :])
            r33 = sb_tmp.tile([1, 512], dt.float32, tag="r33")
            nc.vector.reciprocal_approx_fast(out=r33, in_=s33[0:1, :])
            r32g = sb_tmp.tile([1, 512], dt.float32, tag="r32g")
            nc.vector.tensor_scalar_mul(r32g, r32, gamma2)
            r33g = sb_tmp.tile([1, 512], dt.float32, tag="r33g")
            nc.vector.tensor_scalar_mul(r33g, r33, gamma3)
            # broadcast gamma/denominator across partitions with a pair of
            # DMAs through a DRAM bounce (stride-0 partition reads are only
            # legal from DRAM) -- no TensorE or PSUM involvement
            rb32 = nc.dram_tensor(f"rb32_{mc}", [1, 512], dt.float32)
            nc.gpsimd.dma_start(out=rb32[:, :], in_=r32g)
            rb33 = nc.dram_tensor(f"rb33_{mc}", [1, 512], dt.float32)
            nc.gpsimd.dma_start(out=rb33[:, :], in_=r33g)
            b32 = sb_tmp.tile([KA, 512], dt.float32, tag="b32")
            nc.gpsimd.dma_start(out=b32, in_=rb32[0:1, :].to_broadcast((KA, 512)))
            b33 = sb_tmp.tile([KA, 512], dt.float32, tag="b33")
            nc.gpsimd.dma_start(out=b33, in_=rb33[0:1, :].to_broadcast((KA, 512)))
            return (mc, s32, s33, b32, b33)

        def emit_norm(staged):
            mc, s32, s33, b32, b33 = staged
            t32 = sb_tmp.tile([KA, 512], dt.float32, tag="t32")
            nc.vector.tensor_mul(t32, s32, b32)
            t33 = sb_tmp.tile([KA, 512], dt.float32, tag="t33")
            nc.vector.tensor_mul(t33, s33, b33)
            s = sb_tmp.tile([KA, 512], dt.float32, tag="s")
            nc.vector.tensor_add(s, t32, t33)
            o_sb = sb_out.tile([KA, 512], dt.float32, tag="osb")
            nc.vector.tensor_add(o_sb, s, x3res_sb[:, ts(mc, 512)])
            nc.gpsimd.dma_start(out=out_e[:, ts(mc, 512)], in_=o_sb[1 : 1 + CH, :])

        staged = {"cur": None}

        def emit_out_mms(p):
            ex, mc_p, ntl_p = p
            o32, o33 = o_tiles[mc_p]
            nc.tensor.matmul(
                o32, lhsT=v3t[:, ntl_p, :], rhs=ex[:, 0:512],
                start=(ntl_p == 0), stop=(ntl_p == NT - 1),
            )
            nc.tensor.matmul(
                o33, lhsT=v3t[:, ntl_p, :], rhs=ex[:, 512:1024],
                start=(ntl_p == 0), stop=(ntl_p == NT - 1),
            )
            if ntl_p == NT - 1:
                staged["cur"] = emit_stage(mc_p)

        for j in range(M_LOC // 512):
            q_chunk(j)
        p_chunk(0)
        p_chunk(1)
        for ntl in range(8):
            v3t_make(ntl)
        pend = None
        for mc in range(MC):
            o32_t = ps_o.tile([KA, 512], dt.float32, tag="o32")
            o33_t = ps_o.tile([KA, 512], dt.float32, tag="o33")
            o_tiles[mc] = (o32_t, o33_t)
            for ntl in range(NT):
                if mc == 0:
                    if ntl % 4 == 2 and 2 <= ntl // 4 + 1 < N // 512:
                        p_chunk(ntl // 4 + 1)   # two steps ahead of first use
                    if ntl < NT - 8:
                        v3t_make(ntl + 8)
                if ntl % 2 == 0:
                    # four K=8 energy matmuls (both maps x two key tiles) run
                    # concurrently in the four 32-row PE strips
                    ea = ps_e.tile([128, 1024], dt.float32, tag="e", name="ea")
                    eb = ps_e.tile([128, 1024], dt.float32, tag="e", name="eb")
                    for g, (ept, nn) in enumerate(
                        ((ea, ntl), (ea, ntl), (eb, ntl + 1), (eb, ntl + 1))
                    ):
                        nc.tensor.matmul(
                            ept[:, 512 * (g % 2) : 512 * (g % 2) + 512],
                            lhsT=p_all[32 * g : 32 * g + D, ts(nn, 128)],
                            rhs=q_all[32 * g : 32 * g + D, ts(mc, 512)],
                            start=True, stop=True, tile_position=(32 * g, 0),
                        )
                    e_pair = (ea, eb)
                e_ps = e_pair[ntl % 2]
                ex = sb_e.tile([128, 1024], dt.bfloat16, tag="ex")
                nc.scalar.activation(out=ex, in_=e_ps, func=EXP)
                if pend is not None:
                    emit_out_mms(pend)
                pend = (ex, mc, ntl)
                if ntl == 8 and staged["cur"] is not None:
                    emit_norm(staged.pop("cur"))
                    staged["cur"] = None
        emit_out_mms(pend)
        emit_norm(staged.pop("cur"))

    nc.compile()
    return nc


_CACHE = {}


def _get_nc(gamma2: float, gamma3: float) -> bass.Bass:
    key = (gamma2, gamma3)
    if key not in _CACHE:
        _CACHE[key] = build(gamma2, gamma3)
    return _CACHE[key]


def prep(x, wq2, bq2, wq3, bq3, wv3, bv3, gamma2, gamma3):
    """Build (nc, in_maps) for the 8-core SPMD launch."""
    x = np.asarray(x, dtype=np.float32)
    B, C, W, H = x.shape
    n = W * H
    ch = C // 2
    assert (B, C, n) == (4, 128, N), (B, C, n)

    g2 = float(np.asarray(gamma2).reshape(-1)[0])
    g3 = float(np.asarray(gamma3).reshape(-1)[0])
    nc = _get_nc(g2, g3)

    wq2bT = np.concatenate(
        [np.asarray(wq2, np.float32).T, np.asarray(bq2, np.float32)[None, :]], axis=0
    ).astype(BF16)
    wq3bT = np.concatenate(
        [np.asarray(wq3, np.float32).T, np.asarray(bq3, np.float32)[None, :]], axis=0
    ).astype(BF16)
    # column 0 selects the ones row of x3aug (softmax denominator); the
    # value/bias columns follow at 1..64
    wv3bT = np.zeros((KA, KA), np.float32)
    wv3bT[CH, 0] = 1.0
    wv3bT[:CH, 1:] = np.asarray(wv3, np.float32).T
    wv3bT[CH, 1:] = np.asarray(bv3, np.float32)
    wv3bT = wv3bT.astype(BF16)

    xf = x.reshape(B, C, n)
    ones = np.ones((1, n), np.float32)
    in_maps = []
    for b in range(B):
        x3 = xf[b, :ch]
        x2 = xf[b, ch:]
        x3aug = np.concatenate([x3, ones], axis=0).astype(BF16)
        x2aug = np.concatenate([x2, ones], axis=0).astype(BF16)
        for h in range(2):
            sl = ts(h, M_LOC)
            in_maps.append(
                {
                    "x3aug": x3aug,
                    "x2aug": x2aug,
                    "x3q": np.ascontiguousarray(x3aug[:, sl]),
                    "x3res": np.ascontiguousarray(x3[:, sl]),
                    "wq2bT": wq2bT,
                    "wq3bT": wq3bT,
                    "wv3bT": wv3bT,
                }
            )

    return nc, in_maps


def gather(outs, B=4, ch=CH, n=N, W=64, H=64):
    out = np.empty((B, ch, n), np.float32)
    for b in range(B):
        for h in range(2):
            out[b, :, ts(h, M_LOC)] = np.asarray(outs[2 * b + h]["out"])
    return out.reshape(B, ch, W, H)


def kernel(**inputs):
    nc, in_maps = prep(**inputs)
    res = run_bass_kernel_spmd(nc, in_maps, core_ids=list(range(8)))
    return gather(res.results)


# revision 14
# speedup vs baseline: 1.0237x; 1.0237x over previous
# Distributed Trainium2 kernel for the dual-map spatial attention module:
#   x3 = x[:, :64], x2 = x[:, 64:]  (as [B, 64, N], N = 64*64 = 4096)
#   p2 = wq2 @ x2 + bq2 ; p3 = wq3 @ x3 + bq3 ; v3 = wv3 @ x3 + bv3
#   att32 = softmax(p3^T p2), att33 = softmax(p3^T p3)   (row softmax over keys)
#   out = gamma2 * (v3 @ att32^T) + gamma3 * (v3 @ att33^T) + x3
#
# Sharding: data-parallel over batch (4) x query-halves (2) -> 8 cores.
# Each core sees the full keys/values of its batch and computes the output
# for its 2048 query columns. No collectives needed.
#
# Per-core dataflow (all attention math in a key-streaming loop):
#   - project p2/p3 keys [8, 4096], queries p3q [8, 2048], and the
#     transposed value matrix v3T [4096, 65] (last column = ones, so the
#     out-matmul also produces the softmax denominator row).
#   - for each 512-wide query chunk, stream 32 key tiles of 128:
#       energy matmuls (K=8) -> PSUM f32, one Exp over [128, 1024]
#       (both attention maps share the activation op) -> bf16 SBUF,
#       immediately consumed by the accumulating out-matmuls (K=128).
#   - epilogue: reciprocal of the denominator row, gamma folded into a
#     rank-1 broadcast matmul, two fused multiplies + residual add.
import sys

if "/opt/trn_rl_repo" not in sys.path:
    sys.path.insert(0, "/opt/trn_rl_repo")

from contextlib import ExitStack

import numpy as np
import ml_dtypes

import concourse.bass as bass
import concourse.tile as tile
from concourse import bacc, mybir
from concourse.bass_utils import run_bass_kernel_spmd

BF16 = ml_dtypes.bfloat16
dt = mybir.dt

N = 4096          # keys per batch (64*64 spatial positions)
M_LOC = 2048      # queries per core (half a batch)
CH = 64           # output channels (c_half)
D = 8             # q/k projection dim
KA = CH + 1       # augmented contraction dim (channels + ones row)
NT = N // 128     # key tiles
MC = M_LOC // 512 # query chunks per core

def ts(i, size):
    return slice(i * size, (i + 1) * size)


def build(gamma2: float, gamma3: float) -> bass.Bass:
    # Bacc (vs raw Bass): its compile() splits multi-semaphore waits into
    # event-semaphore chains, which this walrus build requires (it rejects
    # instructions carrying more than one sync wait).
    nc = bacc.Bacc()

    x3aug = nc.declare_dram_parameter("x3aug", [KA, N], dt.bfloat16, isOutput=False)
    x2aug = nc.declare_dram_parameter("x2aug", [KA, N], dt.bfloat16, isOutput=False)
    x3q = nc.declare_dram_parameter("x3q", [KA, M_LOC], dt.bfloat16, isOutput=False)
    x3res = nc.declare_dram_parameter("x3res", [CH, M_LOC], dt.float32, isOutput=False)
    wq2bT = nc.declare_dram_parameter("wq2bT", [KA, D], dt.bfloat16, isOutput=False)
    wq3bT = nc.declare_dram_parameter("wq3bT", [KA, D], dt.bfloat16, isOutput=False)
    wv3bT = nc.declare_dram_parameter("wv3bT", [KA, KA], dt.bfloat16, isOutput=False)
    out_e = nc.declare_dram_parameter("out", [CH, M_LOC], dt.float32, isOutput=True)

    EXP = mybir.ActivationFunctionType.Exp
    NXC = 4               # x2aug/x3aug arrive in NXC column chunks
    XC = N // NXC

    with ExitStack() as ctx:
        tc = ctx.enter_context(tile.TileContext(nc))
        singles = ctx.enter_context(tc.tile_pool(name="singles", bufs=1))
        ps_e = ctx.enter_context(tc.tile_pool(name="ps_e", bufs=3, space="PSUM"))
        ps_o = ctx.enter_context(tc.tile_pool(name="ps_o", bufs=1, space="PSUM"))
        sb_e = ctx.enter_context(tc.tile_pool(name="sb_e", bufs=3))
        sb_tmp = ctx.enter_context(tc.tile_pool(name="sb_tmp", bufs=2))
        sb_out = ctx.enter_context(tc.tile_pool(name="sb_out", bufs=2))

        # ---- inputs -> SBUF.  Weights first (the projections need them
        # immediately and the HWDGE queue is in-order), then the activations
        # in first-use order.  x2aug/x3aug land as separate column-chunk
        # tiles so the just-in-time projections only wait for their chunk.
        wq2bT_sb = singles.tile([KA, D], dt.bfloat16)
        nc.sync.dma_start(out=wq2bT_sb, in_=wq2bT[:, :])
        wq3bT_sb = singles.tile([KA, D], dt.bfloat16)
        nc.sync.dma_start(out=wq3bT_sb, in_=wq3bT[:, :])
        wv3bT_sb = singles.tile([KA, KA], dt.bfloat16)
        nc.sync.dma_start(out=wv3bT_sb, in_=wv3bT[:, :])
        x3q_sb = singles.tile([KA, M_LOC], dt.bfloat16)
        nc.sync.dma_start(out=x3q_sb, in_=x3q[:, :])

        x2c = []
        x3c = []
        for c in range(NXC):
            t2 = singles.tile([KA, XC], dt.bfloat16, name=f"x2c{c}", tag=f"x2c{c}")
            nc.sync.dma_start(out=t2, in_=x2aug[:, ts(c, XC)])
            x2c.append(t2)
            t3 = singles.tile([KA, XC], dt.bfloat16, name=f"x3c{c}", tag=f"x3c{c}")
            nc.sync.dma_start(out=t3, in_=x3aug[:, ts(c, XC)])
            x3c.append(t3)

        # residual lands on rows 1:65 so every epilogue op is base-partition-0
        x3res_sb = singles.tile([KA, M_LOC], dt.float32)
        nc.vector.memset(x3res_sb[0:1, :], 0.0)
        nc.sync.dma_start(out=x3res_sb[1 : 1 + CH, :], in_=x3res[:, :])

        # gamma rows for the final chunk's PE-side broadcast (the tail has
        # an idle TensorE; the DMA-bounce broadcast would add ~5us there)
        g2row = singles.tile([1, KA], dt.bfloat16)
        nc.vector.memset(g2row, gamma2)
        g3row = singles.tile([1, KA], dt.bfloat16)
        nc.vector.memset(g3row, gamma3)

        # Keys and queries live four times, at partition rows 0:8 / 32:40 /
        # 64:72 / 96:104 (p2, p3, p2, p3).  The four energy matmuls of a
        # step pair then run in the four disjoint 32-row PE strips (row
        # tiles T0/T4/T8/T12) concurrently.
        p_all = singles.tile([128, N], dt.bfloat16)
        q_all = singles.tile([128, M_LOC], dt.bfloat16)
        v3t = singles.tile([128, NT, KA], dt.bfloat16)

        def x_slice(tiles, j):
            # 512-column slice j out of the chunked x tiles
            per = XC // 512
            return tiles[j // per][:, ts(j % per, 512)]

        def proj_chunk(dst, j, lhs0, rhs0, lhs1, rhs1):
            # four column-packed projections into one PSUM tile (rows 0:8 and
            # 64:72 from (lhs0, rhs0); rows 32:40 and 96:104 from
            # (lhs1, rhs1)); one wide cast out.
            pp = ps_e.tile([128, 512], dt.float32, tag="e")
            for g, (lh, rh) in enumerate(
                ((lhs0, rhs0), (lhs1, rhs1), (lhs0, rhs0), (lhs1, rhs1))
            ):
                nc.tensor.matmul(
                    pp[32 * g : 32 * g + D, :], lhsT=lh, rhs=rh,
                    start=True, stop=True, tile_position=(0, 32 * g),
                )
            nc.vector.tensor_copy(out=dst[:, ts(j, 512)], in_=pp)

        def v3t_make(ntl):
            # v3T tile [128 keys, 65] = x3aug_tile^T @ wv3bT; column 0 of
            # wv3bT picks out the ones row -> out-matmul row 0 accumulates
            # the softmax denominator.
            vp = ps_e.tile([128, KA], dt.float32, tag="e")
            per = XC // 128
            nc.tensor.matmul(
                vp, lhsT=x3c[ntl // per][:, ts(ntl % per, 128)], rhs=wv3bT_sb,
                start=True, stop=True,
            )
            nc.vector.tensor_copy(out=v3t[:, ntl, :], in_=vp)

        def q_chunk(j):
            proj_chunk(q_all, j, wq3bT_sb, x3q_sb[:, ts(j, 512)],
                       wq3bT_sb, x3q_sb[:, ts(j, 512)])

        def p_chunk(j):
            proj_chunk(p_all, j, wq2bT_sb, x_slice(x2c, j),
                       wq3bT_sb, x_slice(x3c, j))

        # ---- main attention loop, software-pipelined: the out-matmuls of
        # step i are emitted next to the energy matmuls of step i+1, so the
        # (in-order) PE stream never parks right behind the Exp it feeds.
        # Key-side projections and v3T tiles are produced just-in-time
        # during the first query chunk; later query chunks are produced in
        # the middle of the preceding chunk's loop.
        o_tiles = {}

        def emit_stage(mc, last=False):
            # stage the accumulators out of PSUM quickly (frees the o banks
            # for the next chunk); the normalization itself is deferred a few
            # steps so its PE work never head-of-line-blocks the energy
            # matmuls while the reciprocals run on the vector engine.
            o32, o33 = o_tiles.pop(mc)
            s32 = sb_tmp.tile([KA, 512], dt.float32, tag="s32")
            nc.vector.tensor_copy(out=s32, in_=o32)
            s33 = sb_tmp.tile([KA, 512], dt.float32, tag="s33")
            nc.vector.tensor_copy(out=s33, in_=o33)
            r32 = sb_tmp.tile([1, 512], dt.float32, tag="r32")
            nc.vector.reciprocal_approx_fast(out=r32, in_=s32[0:1, :])
            r33 = sb_tmp.tile([1, 512], dt.float32, tag="r33")
            nc.vector.reciprocal_approx_fast(out=r33, in_=s33[0:1, :])
            if last:
                # tail path: idle TensorE does the partition broadcast (and
                # applies gamma via the g-rows); lower latency than the DMA
                # bounce below
                r32b = sb_tmp.tile([1, 512], dt.bfloat16, tag="r32b")
                nc.vector.tensor_copy(out=r32b, in_=r32)
                r33b = sb_tmp.tile([1, 512], dt.bfloat16, tag="r33b")
                nc.vector.tensor_copy(out=r33b, in_=r33)
                b32p = ps_e.tile([KA, 512], dt.float32, tag="e", name="b32p")
                nc.tensor.matmul(b32p, lhsT=g2row, rhs=r32b, start=True, stop=True)
                b33p = ps_e.tile([KA, 512], dt.float32, tag="e", name="b33p")
                nc.tensor.matmul(b33p, lhsT=g3row, rhs=r33b, start=True, stop=True)
                return (mc, s32, s33, b32p, b33p)
            r32g = sb_tmp.tile([1, 512], dt.float32, tag="r32g")
            nc.vector.tensor_scalar_mul(r32g, r32, gamma2)
            r33g = sb_tmp.tile([1, 512], dt.float32, tag="r33g")
            nc.vector.tensor_scalar_mul(r33g, r33, gamma3)
            # broadcast gamma/denominator across partitions with a pair of
            # DMAs through a DRAM bounce (stride-0 partition reads are only
            # legal from DRAM) -- no TensorE or PSUM involvement
            rb32 = nc.dram_tensor(f"rb32_{mc}", [1, 512], dt.float32)
            nc.gpsimd.dma_start(out=rb32[:, :], in_=r32g)
            rb33 = nc.dram_tensor(f"rb33_{mc}", [1, 512], dt.float32)
            nc.gpsimd.dma_start(out=rb33[:, :], in_=r33g)
            b32 = sb_tmp.tile([KA, 512], dt.float32, tag="b32")
            nc.gpsimd.dma_start(out=b32, in_=rb32[0:1, :].to_broadcast((KA, 512)))
            b33 = sb_tmp.tile([KA, 512], dt.float32, tag="b33")
            nc.gpsimd.dma_start(out=b33, in_=rb33[0:1, :].to_broadcast((KA, 512)))
            return (mc, s32, s33, b32, b33)

        def emit_norm(staged):
            mc, s32, s33, b32, b33 = staged
            t32 = sb_tmp.tile([KA, 512], dt.float32, tag="t32")
            nc.vector.tensor_mul(t32, s32, b32)
            t33 = sb_tmp.tile([KA, 512], dt.float32, tag="t33")
            nc.vector.tensor_mul(t33, s33, b33)
            s = sb_tmp.tile([KA, 512], dt.float32, tag="s")
            nc.vector.tensor_add(s, t32, t33)
            o_sb = sb_out.tile([KA, 512], dt.float32, tag="osb")
            nc.vector.tensor_add(o_sb, s, x3res_sb[:, ts(mc, 512)])
            nc.gpsimd.dma_start(out=out_e[:, ts(mc, 512)], in_=o_sb[1 : 1 + CH, :])

        staged = {"cur": None}

        def emit_out_mms(p):
            ex, mc_p, ntl_p = p
            o32, o33 = o_tiles[mc_p]
            nc.tensor.matmul(
                o32, lhsT=v3t[:, ntl_p, :], rhs=ex[:, 0:512],
                start=(ntl_p == 0), stop=(ntl_p == NT - 1),
            )
            nc.tensor.matmul(
                o33, lhsT=v3t[:, ntl_p, :], rhs=ex[:, 512:1024],
                start=(ntl_p == 0), stop=(ntl_p == NT - 1),
            )
            if ntl_p == NT - 1:
                staged["cur"] = emit_stage(mc_p, last=(mc_p == MC - 1))

        for j in range(M_LOC // 512):
            q_chunk(j)
        p_chunk(0)
        p_chunk(1)
        for ntl in range(8):
            v3t_make(ntl)
        pend = None
        for mc in range(MC):
            o32_t = ps_o.tile([KA, 512], dt.float32, tag="o32")
            o33_t = ps_o.tile([KA, 512], dt.float32, tag="o33")
            o_tiles[mc] = (o32_t, o33_t)
            for ntl in range(NT):
                if mc == 0:
                    if ntl % 4 == 2 and 2 <= ntl // 4 + 1 < N // 512:
                        p_chunk(ntl // 4 + 1)   # two steps ahead of first use
                    if ntl < NT - 8:
                        v3t_make(ntl + 8)
                if ntl % 2 == 0:
                    # four K=8 energy matmuls (both maps x two key tiles) run
                    # concurrently in the four 32-row PE strips
                    ea = ps_e.tile([128, 1024], dt.float32, tag="e", name="ea")
                    eb = ps_e.tile([128, 1024], dt.float32, tag="e", name="eb")
                    for g, (ept, nn) in enumerate(
                        ((ea, ntl), (ea, ntl), (eb, ntl + 1), (eb, ntl + 1))
                    ):
                        nc.tensor.matmul(
                            ept[:, 512 * (g % 2) : 512 * (g % 2) + 512],
                            lhsT=p_all[32 * g : 32 * g + D, ts(nn, 128)],
                            rhs=q_all[32 * g : 32 * g + D, ts(mc, 512)],
                            start=True, stop=True, tile_position=(32 * g, 0),
                        )
                    e_pair = (ea, eb)
                e_ps = e_pair[ntl % 2]
                ex = sb_e.tile([128, 1024], dt.bfloat16, tag="ex")
                nc.scalar.activation(out=ex, in_=e_ps, func=EXP)
                if pend is not None:
                    emit_out_mms(pend)
                pend = (ex, mc, ntl)
                if ntl == 8 and staged["cur"] is not None:
                    emit_norm(staged.pop("cur"))
                    staged["cur"] = None
        emit_out_mms(pend)
        emit_norm(staged.pop("cur"))

    nc.compile()
    return nc


_CACHE = {}


def _get_nc(gamma2: float, gamma3: float) -> bass.Bass:
    key = (gamma2, gamma3)
    if key not in _CACHE:
        _CACHE[key] = build(gamma2, gamma3)
    return _CACHE[key]


def prep(x, wq2, bq2, wq3, bq3, wv3, bv3, gamma2, gamma3):
    """Build (nc, in_maps) for the 8-core SPMD launch."""
    x = np.asarray(x, dtype=np.float32)
    B, C, W, H = x.shape
    n = W * H
    ch = C // 2
    assert (B, C, n) == (4, 128, N), (B, C, n)

    g2 = float(np.asarray(gamma2).reshape(-1)[0])
    g3 = float(np.asarray(gamma3).reshape(-1)[0])
    nc = _get_nc(g2, g3)

    wq2bT = np.concatenate(
        [np.asarray(wq2, np.float32).T, np.asarray(bq2, np.float32)[None, :]], axis=0
    ).astype(BF16)
    wq3bT = np.concatenate(
        [np.asarray(wq3, np.float32).T, np.asarray(bq3, np.float32)[None, :]], axis=0
    ).astype(BF16)
    # column 0 selects the ones row of x3aug (softmax denominator); the
    # value/bias columns follow at 1..64
    wv3bT = np.zeros((KA, KA), np.float32)
    wv3bT[CH, 0] = 1.0
    wv3bT[:CH, 1:] = np.asarray(wv3, np.float32).T
    wv3bT[CH, 1:] = np.asarray(bv3, np.float32)
    wv3bT = wv3bT.astype(BF16)

    xf = x.reshape(B, C, n)
    ones = np.ones((1, n), np.float32)
    in_maps = []
    for b in range(B):
        x3 = xf[b, :ch]
        x2 = xf[b, ch:]
        x3aug = np.concatenate([x3, ones], axis=0).astype(BF16)
        x2aug = np.concatenate([x2, ones], axis=0).astype(BF16)
        for h in range(2):
            sl = ts(h, M_LOC)
            in_maps.append(
                {
                    "x3aug": x3aug,
                    "x2aug": x2aug,
                    "x3q": np.ascontiguousarray(x3aug[:, sl]),
                    "x3res": np.ascontiguousarray(x3[:, sl]),
                    "wq2bT": wq2bT,
                    "wq3bT": wq3bT,
                    "wv3bT": wv3bT,
                }
            )

    return nc, in_maps


def gather(outs, B=4, ch=CH, n=N, W=64, H=64):
    out = np.empty((B, ch, n), np.float32)
    for b in range(B):
        for h in range(2):
            out[b, :, ts(h, M_LOC)] = np.asarray(outs[2 * b + h]["out"])
    return out.reshape(B, ch, W, H)


def kernel(**inputs):
    nc, in_maps = prep(**inputs)
    res = run_bass_kernel_spmd(nc, in_maps, core_ids=list(range(8)))
    return gather(res.results)
